# revision 31
# baseline (speedup 1.0000x reference)
"""Causal GQA attention (nkv=1) with RoPE + logit softcap, sharded over 8 trn2 cores.

Sharding: core = 2*b + hh  (b = batch 0..3, hh = head-half 0..1).
Each core computes, for its batch b and its 4 query heads:
  q = rope(x @ Wq_h'.T)          (gain/(sqrt(hd)*softcap) folded into Wq on host)
  k = rope(x @ Wk.T), v = x @ Wv.T   (single kv head, shared across its 4 q heads)
  pT[k,q] = exp(softcap*tanh(qT.k + causal_bias))   (max-free softmax: softcap
            bounds logits, masked entries get a -8 pre-tanh bias -> e^-30 ~ 0)
  outT_h = (v.T @ pT) / sum_k pT    accumulated in PSUM; denominator via ones-matmul
  partial_out[tok, :] = sum_h outT_h.T @ Wo[:, head cols].T
Host sums the two half-head partials per batch and stacks batches.

v5.3 = v3's ascending-chunk schedule (measured near-zero mid-kernel ACT
idle; v4's reverse-chunk experiments all lost more in the middle than they
saved at the ends) with both ends fixed:
  - head: first task (0,0,0) needs only x(c0)+wk+wq_h0+rope(c0)+mask consts;
    the DMA queues prioritize exactly that prefix, the ACT table set
    preloads via a dummy tanh+exp during the DMA wait, chunk 0-2 K-ropes
    run on DVE (gpsimd is ~4us slower per rope chain), the kp(0)/qp(0,0)
    PSUM casts ride the pre-first-tanh-idle ACT engine, and chunk 0 is
    hand-interleaved (projection h+1 right after scores h) so the four diag
    tasks stream at the DVE rope cadence. First tanh ~18us (v3: ~27us).
  - causal mask folded into the diagonal score accumulation as a -8 bias
    matmul (stationary identity, moving bias pattern): kills 64 DVE muls
    (measured up to ~1.2us each in-place) at +3.4us of N=128 PE matmuls.
  - Wq shipped per-head contiguous ([NHL*128, NKT*HD]) so per-head slices
    DMA as 2KB lines instead of 256B segments.
  - tail: chunk-3 Wo is split by heads: h0+h1 accumulate into fp32 SBUF
    partials as late chunk-3 fillers (their avn are ready mid-chunk); after
    the last exp only the h2+h3 matmuls + a DVE add + split-queue DMA
    remain. Both 512-col output halves of a token block stage into one
    [128,1024] tile and DMA as a single full-row transfer (2KB lines; the
    1KB version left the final writeback packet-rate-bound at ~110GB/s).
    AV lag drops to 1 in chunk 3. ~8.5us of the remaining tail is the
    fixed per-semaphore-zeroing engine postamble (framework-emitted).
All matmuls bf16 (1 cyc/row); scores accumulate fp32 in PSUM; tanh keeps
fp32 until the bf16 exp output. Known-failed: fp8/DoubleRow projections
(even one fp8 tensor exceeds the 2e-2 max-rel-err gate), gpsimd
quad-compress or any gpsimd op feeding a PE matmul (its latency stalls the
in-order PE FIFO), reverse chunk order, fine-grained cost-paced fillers,
PE warmup matmuls, strided wq slices on the scalar DMA queue (256B-segment
transfers clog the queue and delay the rope tables).
"""
import numpy as np
import ml_dtypes

import concourse.bacc as bacc
import concourse.mybir as mybir
import concourse.tile as tile
from concourse.bass_utils import run_bass_kernel_spmd

F32 = mybir.dt.float32
BF16 = mybir.dt.bfloat16
NPBF16 = ml_dtypes.bfloat16

B, T, D = 4, 2048, 1024
NH, NKV, HD = 8, 1, 128
SOFTCAP = 30.0
NHL = 4            # heads per core
CH = 512           # q-chunk size
NCH = T // CH      # 4 chunks
NKT = D // 128     # 8 k-tiles over D
NTT = T // 128     # 16 token tiles

# packed column offsets for diagonal groups: k-block j (visible width
# 512-128j) starts at DOFF[j], arranged so every matmul output stays inside
# one 2KB PSUM bank (512 fp32) with zero padding: bank0 = j0(512),
# bank1 = j1(384) + j3(128), bank2 = j2(256). Total 1280 packed columns.
DOFF = [0, 512, 1024, 896]
NWD = 1280         # diag tanh/exp span
DW = 4 * CH        # score tile width (non-diag)


def _build_nc():
    nc = bacc.Bacc()

    xT = nc.dram_tensor("xT", [D, T], BF16, kind="ExternalInput")
    wqT = nc.dram_tensor("wqT", [NHL * 128, NKT * HD], BF16,
                         kind="ExternalInput")
    wkT = nc.dram_tensor("wkT", [D, HD], BF16, kind="ExternalInput")
    wvT = nc.dram_tensor("wvT", [D, HD], BF16, kind="ExternalInput")
    woT = nc.dram_tensor("woT", [NHL * HD, D], BF16, kind="ExternalInput")
    cc = nc.dram_tensor("cc", [HD, T], BF16, kind="ExternalInput")
    ssw = nc.dram_tensor("ssw", [HD, T], BF16, kind="ExternalInput")
    trib = nc.dram_tensor("trib", [128, 128], BF16, kind="ExternalInput")
    idm = nc.dram_tensor("idm", [128, 128], BF16, kind="ExternalInput")
    onesv = nc.dram_tensor("onesv", [128, 128], BF16, kind="ExternalInput")
    out = nc.dram_tensor("out", [T, D], BF16, kind="ExternalOutput")

    xT_t = xT.rearrange("(kt p) t -> p kt t", p=128)      # [128, 8, 2048]
    wqT_t = wqT.rearrange("(h p) (kt c) -> p h kt c", p=128,
                          kt=NKT)                         # [128, 4, 8, 128]
    wkT_t = wkT.rearrange("(kt p) c -> p kt c", p=128)    # [128, 8, 128]
    wvT_t = wvT.rearrange("(kt p) c -> p kt c", p=128)    # [128, 8, 128]
    woT_t = woT.rearrange("(h p) c -> p h c", p=128)      # [128, 4, 1024]

    with tile.TileContext(nc) as tc:
        with (
            tc.tile_pool(name="persist", bufs=1) as persist,
            tc.tile_pool(name="wpool", bufs=1) as wpool,
            tc.tile_pool(name="qt_pool", bufs=2) as qt_pool,
            tc.tile_pool(name="rope_pool", bufs=3) as rope_pool,
            tc.tile_pool(name="p_pool", bufs=5) as p_pool,
            tc.tile_pool(name="pp_pool", bufs=4) as pp_pool,
            tc.tile_pool(name="t4_pool", bufs=1) as t4_pool,
            tc.tile_pool(name="avn_pool", bufs=12) as avn_pool,
            tc.tile_pool(name="osb_pool", bufs=4) as osb_pool,
            tc.tile_pool(name="o32_pool", bufs=8) as o32_pool,
            tc.tile_pool(name="norm_pool", bufs=2) as norm_pool,
            tc.tile_pool(name="s_pool", bufs=1, space="PSUM") as s_pool,
            tc.tile_pool(name="acc_pool", bufs=1, space="PSUM") as acc_pool,
            tc.tile_pool(name="d_pool", bufs=1, space="PSUM") as d_pool,
            tc.tile_pool(name="pj_pool", bufs=2, space="PSUM") as pj_pool,
        ):
            # --- persistent tiles ---
            wq_sb = wpool.tile([128, NHL, NKT, HD], BF16)
            wk_sb = wpool.tile([128, NKT, HD], BF16)
            wv_sb = wpool.tile([128, NKT, HD], BF16)
            wo_sb = wpool.tile([128, NHL, D], BF16)
            cc_sb = wpool.tile([HD, T], BF16)
            ssw_sb = wpool.tile([HD, T], BF16)
            trib_sb = wpool.tile([128, 128], BF16)
            idm_sb = wpool.tile([128, 128], BF16)
            ones_sb = wpool.tile([128, 128], BF16)
            xT_sb = wpool.tile([128, NKT, T], BF16)
            kT_sb = persist.tile([HD, T], BF16)
            v_sb = persist.tile([128, NTT, HD], BF16)

            # ACT table preload: dummy tanh+exp (same table set) so the
            # ~2.7us ACT_TABLE_LOAD runs during the DMA wait, not inside
            # the first real tanh.
            warm = wpool.tile([1, 16], F32)
            nc.vector.memset(warm[:], 0.0)
            warm2 = wpool.tile([1, 16], F32)
            nc.scalar.activation(warm2[:], warm[:],
                                 mybir.ActivationFunctionType.Tanh)
            nc.scalar.activation(warm2[:], warm[:],
                                 mybir.ActivationFunctionType.Exp)

            # --- DMA priorities ---
            # First task (0,0,0) needs x(c0), wk, wq-h0, cc/ssw(c0) and the
            # mask consts; everything else follows in chunk order. The two
            # HWDGE queues stripe over the same 16 engines, so the critical
            # prefix is split across both; the scalar queue finishes its
            # issues by ~12us so the Scalar engine is pure ACT afterwards.
            c0 = slice(0, CH)
            c1 = slice(CH, 2 * CH)
            c2 = slice(2 * CH, 3 * CH)
            c3 = slice(3 * CH, T)
            nc.sync.dma_start(xT_sb[:, 0:1, c0], xT_t[:, 0:1, c0])
            nc.scalar.dma_start(wk_sb[:], wkT_t)
            nc.sync.dma_start(xT_sb[:, 1:4, c0], xT_t[:, 1:4, c0])
            nc.scalar.dma_start(cc_sb[:, c0], cc[:, c0])
            nc.scalar.dma_start(ssw_sb[:, c0], ssw[:, c0])
            nc.sync.dma_start(xT_sb[:, 4:8, c0], xT_t[:, 4:8, c0])
            nc.scalar.dma_start(trib_sb[:], trib[:])
            nc.scalar.dma_start(idm_sb[:], idm[:])
            nc.scalar.dma_start(ones_sb[:], onesv[:])
            nc.sync.dma_start(wq_sb[:, 0], wqT_t[:, 0])
            nc.scalar.dma_start(wv_sb[:], wvT_t)
            nc.scalar.dma_start(cc_sb[:, c1], cc[:, c1])
            nc.scalar.dma_start(ssw_sb[:, c1], ssw[:, c1])
            nc.sync.dma_start(xT_sb[:, :, c1], xT_t[:, :, c1])
            nc.sync.dma_start(wq_sb[:, 1], wqT_t[:, 1])
            nc.sync.dma_start(xT_sb[:, :, c2], xT_t[:, :, c2])
            nc.sync.dma_start(wq_sb[:, 2], wqT_t[:, 2])
            nc.sync.dma_start(wq_sb[:, 3], wqT_t[:, 3])
            nc.sync.dma_start(cc_sb[:, c2], cc[:, c2])
            nc.sync.dma_start(ssw_sb[:, c2], ssw[:, c2])
            nc.sync.dma_start(xT_sb[:, :, c3], xT_t[:, :, c3])
            nc.sync.dma_start(cc_sb[:, c3], cc[:, c3])
            nc.sync.dma_start(ssw_sb[:, c3], ssw[:, c3])
            nc.sync.dma_start(wo_sb[:], woT_t)

            def rope_to(dst_ap, src_ps, c, eng=None, cast_eng=None):
                """dst = rope(src) for a [128, CH] chunk at token offset c*CH.

                Partition half-swap must go through tensor_copy (TT ops need
                aligned partitions). cast_eng handles the PSUM fp32 -> bf16
                read (DVE normally; ACT for the pre-first-tanh units), eng
                (DVE for latency-critical Q/chunk-0-K ropes, gpsimd for the
                pre-emitted later K ropes) runs the mul/add."""
                if eng is None:
                    eng = nc.vector
                csl = slice(c * CH, (c + 1) * CH)
                qb = rope_pool.tile([128, CH], BF16, tag="qb", name="qb")
                if cast_eng is nc.scalar:
                    nc.scalar.copy(qb[:], src_ps[:])
                else:
                    nc.vector.tensor_copy(qb[:], src_ps[:])
                swp = rope_pool.tile([128, CH], BF16, tag="swp", name="swp")
                nc.vector.tensor_copy(swp[0:64, :], qb[64:128, :])
                nc.vector.tensor_copy(swp[64:128, :], qb[0:64, :])
                m1 = rope_pool.tile([128, CH], BF16, tag="m1", name="m1")
                eng.tensor_mul(m1[:], qb[:], cc_sb[:, csl])
                m2 = rope_pool.tile([128, CH], BF16, tag="m2", name="m2")
                eng.tensor_mul(m2[:], swp[:], ssw_sb[:, csl])
                eng.tensor_add(dst_ap, m1[:], m2[:])

            # ---- filler units (pure-PE work scheduled into ACT-bound gaps) ----
            qt_tiles = {}     # c -> qt tile [HD, NHL, CH]

            def qp_unit(c, h):
                csl = slice(c * CH, (c + 1) * CH)
                if h == 0:
                    qt_tiles[c] = qt_pool.tile([HD, NHL, CH], BF16, tag="qt",
                                               name="qt")
                q_ps = pj_pool.tile([128, CH], F32, tag="pj", name="q_ps")
                for kt in range(NKT):
                    nc.tensor.matmul(
                        q_ps[0:HD, :], wq_sb[:, h, kt, :],
                        xT_sb[:, kt, csl], start=(kt == 0), stop=(kt == NKT - 1))
                cast_eng = nc.scalar if (c == 0 and h == 0) else None
                rope_to(qt_tiles[c][:, h, :], q_ps[0:HD, :], c,
                        cast_eng=cast_eng)

            def kp_unit(c):
                csl = slice(c * CH, (c + 1) * CH)
                k_ps = pj_pool.tile([128, CH], F32, tag="pj", name="k_ps")
                for kt in range(NKT):
                    nc.tensor.matmul(k_ps[0:HD, :], wk_sb[:, kt, :],
                                     xT_sb[:, kt, csl],
                                     start=(kt == 0), stop=(kt == NKT - 1))
                # K ropes gate the next chunk's first tasks at each chunk
                # boundary; they stay on DVE (chunk-0's with an ACT cast,
                # pre-first-tanh) except kp(3), emitted ~a chunk ahead,
                # where gpsimd's slowness is hidden.
                if c == 0:
                    rope_to(kT_sb[:, csl], k_ps[0:HD, :], c,
                            cast_eng=nc.scalar)
                else:
                    rope_to(kT_sb[:, csl], k_ps[0:HD, :], c, eng=nc.gpsimd)

            def vp_unit(c, tt):
                # V directly as [tok, hd]: x-tile stationary, wv moving.
                tsl = slice((c * 4 + tt) * 128, (c * 4 + tt + 1) * 128)
                v_ps = pj_pool.tile([128, CH], F32, tag="pj", name="v_ps")
                for kt in range(NKT):
                    nc.tensor.matmul(v_ps[:, 0:HD], xT_sb[:, kt, tsl],
                                     wv_sb[:, kt, :],
                                     start=(kt == 0), stop=(kt == NKT - 1))
                nc.vector.tensor_copy(v_sb[:, c * 4 + tt, :], v_ps[:, 0:HD])

            avn_tiles = {}    # (c, h) -> avn tile
            o32_tiles = {}    # u -> chunk-3 h0+h1 fp32 partial (SBUF)
            osb_pairs = {}    # (c, tt) -> [128, 2*CH] staging tile

            def _osb(c, tt, dc):
                # the two 512-col halves of a token block stage into ONE
                # [128, 1024] tile; the odd half issues a single full-row
                # DMA (2KB lines: the 1KB-line version left the final
                # writeback packet-rate-bound at ~110GB/s)
                if dc == 0:
                    osb_pairs[(c, tt)] = osb_pool.tile(
                        [128, 2 * CH], BF16, tag="osb", name="o_sb")
                return osb_pairs[(c, tt)]

            def wo_unit(c, u):
                tt, dc = u // 2, u % 2
                o_ps = pj_pool.tile([128, CH], F32, tag="pj", name="o_ps")
                for h in range(NHL):
                    nc.tensor.matmul(
                        o_ps[:], avn_tiles[(c, h)][:, tt * 128:(tt + 1) * 128],
                        wo_sb[:, h, dc * CH:(dc + 1) * CH],
                        start=(h == 0), stop=(h == NHL - 1))
                # DMA can't source PSUM (nor can gpsimd): stage on DVE.
                # bf16 halves the out DMA; host sums partials in fp32.
                o_sb = _osb(c, tt, dc)
                nc.vector.tensor_copy(o_sb[:, dc * CH:(dc + 1) * CH], o_ps[:])
                if dc == 1:
                    nc.sync.dma_start(
                        out[c * CH + tt * 128: c * CH + (tt + 1) * 128, :],
                        o_sb[:])

            def wo3a_unit(u):
                # chunk-3 Wo, heads 0+1 only: accumulate into an fp32 SBUF
                # partial during chunk 3 (avn(3,0/1) are ready mid-chunk),
                # so the post-last-exp tail holds only the h2+h3 half.
                tt, dc = u // 2, u % 2
                o_ps = pj_pool.tile([128, CH], F32, tag="pj", name="o_ps")
                for h in (0, 1):
                    nc.tensor.matmul(
                        o_ps[:], avn_tiles[(3, h)][:, tt * 128:(tt + 1) * 128],
                        wo_sb[:, h, dc * CH:(dc + 1) * CH],
                        start=(h == 0), stop=(h == 1))
                o32 = o32_pool.tile([128, CH], F32, tag="o32", name="o32")
                nc.vector.tensor_copy(o32[:], o_ps[:])
                o32_tiles[u] = o32

            def wo3b_unit(u):
                tt, dc = u // 2, u % 2
                o_ps = pj_pool.tile([128, CH], F32, tag="pj", name="o_ps")
                for h in (2, 3):
                    nc.tensor.matmul(
                        o_ps[:], avn_tiles[(3, h)][:, tt * 128:(tt + 1) * 128],
                        wo_sb[:, h, dc * CH:(dc + 1) * CH],
                        start=(h == 2), stop=(h == 3))
                o_sb = _osb(3, tt, dc)
                # fp32 partial + fp32 PSUM -> bf16 out, one DVE add; the
                # tail splits the full-row DMAs across both queues.
                nc.vector.tensor_add(o_sb[:, dc * CH:(dc + 1) * CH], o_ps[:],
                                     o32_tiles[u][:])
                if dc == 1:
                    eng = nc.scalar if tt % 2 == 1 else nc.sync
                    eng.dma_start(
                        out[3 * CH + tt * 128: 3 * CH + (tt + 1) * 128, :],
                        o_sb[:])

            # ---- filler scheduling ----
            emitted = set()

            def emit_unit(u):
                if u in emitted:
                    return
                emitted.add(u)
                kind = u[0]
                if kind == "qp":
                    qp_unit(u[1], u[2])
                elif kind == "kp":
                    kp_unit(u[1])
                elif kind == "vp":
                    vp_unit(u[1], u[2])
                elif kind == "wo":
                    wo_unit(u[1], u[2])
                elif kind == "wo3a":
                    wo3a_unit(u[1])
                elif kind == "wo3b":
                    wo3b_unit(u[1])

            # per-chunk filler lists. Only qp(c+1,0)/kp(c+1) cross chunk
            # boundaries; vp(c) and qp(c,h>=1) stay inside chunk c (forced
            # just-in-time), and Wo shifts late into the ACT-heavy chunks
            # 2 and 3 to match the causal skew of attention work. wo3a units
            # sit at the end of chunk 3's list, after avn(3,0/1) exist.
            fillers = {}
            fillers[0] = ([("qp", 0, 2), ("qp", 0, 3), ("kp", 1)]
                          + [("vp", 0, tt) for tt in range(4)]
                          + [("qp", 1, 0)])
            fillers[1] = ([("vp", 1, tt) for tt in range(4)]
                          + [("kp", 2), ("qp", 2, 0)])
            fillers[2] = ([("vp", 2, tt) for tt in range(4)]
                          + [("wo", 0, u) for u in range(8)]
                          + [("kp", 3), ("qp", 3, 0)])
            fillers[3] = ([("vp", 3, tt) for tt in range(4)]
                          + [("wo", 1, u) for u in range(8)]
                          + [("wo", 2, u) for u in range(8)]
                          + [("wo3a", u) for u in range(8)])

            # ---- attention task machinery ----
            pend = []       # lagged AV work queue: (c, h, g, p4_tile)
            AV_LAG = 2      # tasks between exp(i) and its AV consumption
            prefill = {}    # (c, h, g) -> p4 computed early (ACT-idle fill)
            head_acc = {}   # (c, h) -> (av_ps, d_ps), allocated at g == 0

            def emit_av(c, h, g, p4):
                """AV + quad-compress + ones-matmul for task (c,h,g); the
                consuming accumulators live across the head's groups."""
                diag = g == c
                for tt in range(4):
                    emit_unit(("vp", g, tt))
                if g == 0:
                    av_ps = acc_pool.tile([HD, CH], F32, tag="av", name="av_ps")
                    d_ps = d_pool.tile([128, CH], F32, tag="d", name="d_ps")
                    head_acc[(c, h)] = (av_ps, d_ps)
                av_ps, d_ps = head_acc[(c, h)]
                for j in range(4):
                    kb = 4 * g + j
                    if diag:
                        lo, po = 128 * j, DOFF[j]
                        w = CH - lo
                        nc.tensor.matmul(av_ps[:, lo:CH], v_sb[:, kb, :],
                                         p4[:, po:po + w],
                                         start=(kb == 0),
                                         stop=(g == c and j == 3))
                    else:
                        nc.tensor.matmul(av_ps[:], v_sb[:, kb, :],
                                         p4[:, j * CH:(j + 1) * CH],
                                         start=(kb == 0), stop=False)
                # quad-compress for the denominator: 3 adds -> 1 ones-MM
                ppq = pp_pool.tile([128, CH], BF16, tag="ppq", name="ppq")
                if diag:
                    nc.vector.tensor_copy(ppq[:], p4[:, 0:CH])
                    for j in range(1, 4):
                        lo = 128 * j
                        nc.vector.tensor_add(
                            ppq[:, lo:CH], ppq[:, lo:CH],
                            p4[:, DOFF[j]:DOFF[j] + (CH - lo)])
                else:
                    ppa = pp_pool.tile([128, CH], BF16, tag="ppa", name="ppa")
                    nc.vector.tensor_add(ppa[:], p4[:, 0:CH], p4[:, CH:2 * CH])
                    ppb = pp_pool.tile([128, CH], BF16, tag="ppb", name="ppb")
                    nc.vector.tensor_add(ppb[:], p4[:, 2 * CH:3 * CH],
                                         p4[:, 3 * CH:4 * CH])
                    nc.vector.tensor_add(ppq[:], ppa[:], ppb[:])
                nc.tensor.matmul(d_ps[:], ones_sb[:], ppq[:],
                                 start=(g == 0), stop=(g == c))
                if g == c:
                    # head (c,h) complete: normalize
                    dinv = norm_pool.tile([128, CH], F32, tag="dinv",
                                          name="dinv")
                    nc.vector.reciprocal_approx_fast(dinv[:], d_ps[:])
                    avn = avn_pool.tile([HD, CH], BF16, tag="avn", name="avn")
                    nc.vector.tensor_mul(avn[:], av_ps[:], dinv[:])
                    avn_tiles[(c, h)] = avn

            def emit_scores(c, h, g):
                """scores (+ causal bias on diag) -> tanh -> exp."""
                diag = g == c
                emit_unit(("kp", g))
                qt = qt_tiles[c]
                s_t = s_pool.tile([128, DW], F32, tag="s", name="s_t")
                t4 = t4_pool.tile([128, DW], F32, tag="t4", name="t4")
                p4 = p_pool.tile([128, DW], BF16, tag="p4", name="p4")
                if diag:
                    # causal mask folded into the score accumulation: a -8
                    # bias matmul (stationary identity, moving trib) onto
                    # the first 128 q-cols of each block saturates tanh to
                    # -1, so masked weights exp to e^-30 ~ 0: no DVE muls.
                    for j in range(4):
                        kb = 4 * g + j
                        lo, po = 128 * j, DOFF[j]
                        w = CH - lo
                        nc.tensor.matmul(
                            s_t[:, po:po + w],
                            kT_sb[:, kb * 128:(kb + 1) * 128],
                            qt[:, h, lo:CH], start=True, stop=False)
                        nc.tensor.matmul(
                            s_t[:, po:po + 128], idm_sb[:], trib_sb[:],
                            start=False, stop=True, skip_group_check=True)
                    nw = NWD
                else:
                    for j in range(4):
                        kb = 4 * g + j
                        nc.tensor.matmul(
                            s_t[:, j * CH:(j + 1) * CH],
                            kT_sb[:, kb * 128:(kb + 1) * 128],
                            qt[:, h, :], start=True, stop=True)
                    nw = DW
                nc.scalar.activation(t4[:, 0:nw], s_t[:, 0:nw],
                                     mybir.ActivationFunctionType.Tanh)
                nc.scalar.activation(p4[:, 0:nw], t4[:, 0:nw],
                                     mybir.ActivationFunctionType.Exp,
                                     scale=SOFTCAP)
                return p4

            # ---- main schedule ----
            # prologue: just enough for the first task. K first: wk lands
            # early on the scalar queue, so the PE clock ramp starts earlier.
            emit_unit(("kp", 0))
            emit_unit(("qp", 0, 0))

            # chunk 0 hand-ordered: each next head's projection enters the
            # PE FIFO right after the previous head's scores, so the four
            # diag tasks stream at the DVE rope cadence with no filler
            # bundle in front of the next scores.
            c0units = [[("qp", 0, 1)], [("qp", 0, 2)], [("qp", 0, 3)],
                       [("kp", 1), ("vp", 0, 0), ("vp", 0, 1)]]
            for h in range(NHL):
                p4 = emit_scores(0, h, 0)
                pend.append((0, h, 0, p4))
                while len(pend) > AV_LAG:
                    emit_av(*pend.pop(0))
                for u in c0units[h]:
                    emit_unit(u)
            emit_unit(("qp", 1, 0))
            prefill[(1, 0, 0)] = emit_scores(1, 0, 0)

            for c in range(1, NCH):
                if c >= 1:
                    # cross-boundary fillers must have landed (kp(c)/qp(c,0))
                    for u in fillers[c - 1]:
                        emit_unit(u)
                flist = fillers[c]
                # drain fillers one task early so chunk boundaries are clean
                ntasks = max(1, NHL * (c + 1) - 1)
                nf = len(flist)
                ti = 0
                lag = AV_LAG if c < NCH - 1 else 1
                for h in range(NHL):
                    emit_unit(("qp", c, h))
                    for g in range(c + 1):
                        if (c, h, g) in prefill:
                            p4 = prefill.pop((c, h, g))
                        else:
                            p4 = emit_scores(c, h, g)
                        pend.append((c, h, g, p4))
                        while len(pend) > lag:
                            emit_av(*pend.pop(0))
                        if g == 0 and h + 1 < NHL:
                            # project the next head now: its rope latency
                            # hides under this head's ACT work
                            emit_unit(("qp", c, h + 1))
                        # spread this chunk's fillers evenly across tasks
                        lo = min(nf, (ti * nf) // ntasks)
                        hi = min(nf, ((ti + 1) * nf) // ntasks)
                        for u in flist[lo:hi]:
                            emit_unit(u)
                        ti += 1
                # chunks 0/1 end ACT-idle (proj-heavy, little attention):
                # prefill the next chunk's first non-diag scores/tanh/exp
                # there; their AV stays in place (p4 carries across)
                if c + 1 < NCH and c <= 1:
                    emit_unit(("qp", c + 1, 0))
                    for g in range(min(2, c + 1)):
                        prefill[(c + 1, 0, g)] = emit_scores(c + 1, 0, g)
            while pend:
                emit_av(*pend.pop(0))
            for u in fillers[NCH - 1]:
                emit_unit(u)
            for u in range(8):
                emit_unit(("wo3b", u))

    nc.compile()
    return nc


_CACHED_NC = None


def _get_nc():
    global _CACHED_NC
    if _CACHED_NC is None:
        _CACHED_NC = _build_nc()
    return _CACHED_NC


def _host_inputs(x, Wq, Wk, Wv, Wo, qk_gain, cos, sin):
    """Build the 8 per-core input maps (bf16 matmul operands)."""
    x = np.asarray(x, np.float32)
    Wq = np.asarray(Wq, np.float32)
    Wk = np.asarray(Wk, np.float32)
    Wv = np.asarray(Wv, np.float32)
    Wo = np.asarray(Wo, np.float32)
    qk_gain = np.asarray(qk_gain, np.float32)
    cos = np.asarray(cos, np.float32)
    sin = np.asarray(sin, np.float32)

    scale = 1.0 / (np.sqrt(HD) * SOFTCAP)
    # Fold per-head gain and softcap scale into Wq rows.
    Wq_s = Wq * (qk_gain[:, None].repeat(HD, 1).reshape(NH * HD, 1) * scale)

    wkT = np.ascontiguousarray(Wk.T.astype(NPBF16))
    wvT = np.ascontiguousarray(Wv.T.astype(NPBF16))
    cosT = cos.T  # [64, T]
    sinT = sin.T
    cc = np.ascontiguousarray(np.concatenate([cosT, cosT], 0).astype(NPBF16))
    # m2 = swap(q) * ssw with swap done via copies: ssw = [-sin; sin]
    ssw = np.ascontiguousarray(np.concatenate([-sinT, sinT], 0).astype(NPBF16))

    # causal bias for the diagonal 128-blocks: 0 where visible (qq >= kk),
    # -8 where masked -- saturates tanh pre-exp so masked weights ~ e^-30.
    kk = np.arange(128)
    trib = np.where(kk[None, :] >= kk[:, None], 0.0, -8.0).astype(NPBF16)
    idm = np.eye(128, dtype=NPBF16)
    onesv = np.ones((128, 128), NPBF16)

    xTs = [np.ascontiguousarray(x[b].T.astype(NPBF16)) for b in range(B)]
    in_maps = []
    for core in range(8):
        b, hh = divmod(core, 2)
        h0 = hh * NHL
        # per head h: [p, kt, c] with (kt,c) contiguous per partition row
        wqT = np.ascontiguousarray(np.stack(
            [Wq_s[(h0 + h) * HD:(h0 + h + 1) * HD, :]
             .reshape(HD, NKT, 128).transpose(2, 1, 0).reshape(128, NKT * HD)
             for h in range(NHL)], axis=0)
            .reshape(NHL * 128, NKT * HD).astype(NPBF16))
        woT = np.ascontiguousarray(
            Wo[:, h0 * HD:(h0 + NHL) * HD].T.astype(NPBF16))
        in_maps.append({
            "xT": xTs[b], "wqT": wqT, "wkT": wkT, "wvT": wvT, "woT": woT,
            "cc": cc, "ssw": ssw, "trib": trib, "idm": idm,
            "onesv": onesv,
        })
    return in_maps


def kernel(x, Wq, Wk, Wv, Wo, qk_gain, cos, sin, _trace=False):
    in_maps = _host_inputs(x, Wq, Wk, Wv, Wo, qk_gain, cos, sin)
    nc = _get_nc()
    res = run_bass_kernel_spmd(nc, in_maps, core_ids=list(range(8)),
                               trace=_trace)
    out = np.empty((B, T, D), np.float32)
    for b in range(B):
        out[b] = (res.results[2 * b]["out"].astype(np.float32)
                  + res.results[2 * b + 1]["out"].astype(np.float32))
    if _trace:
        kernel.last_exec_time_ns = res.exec_time_ns
        kernel.last_results = res
    return out


# revision 33
# speedup vs baseline: 1.0263x; 1.0263x over previous
"""Causal GQA attention (nkv=1) with RoPE + logit softcap, sharded over 8 trn2 cores.

Sharding: core = 2*b + hh  (b = batch 0..3, hh = head-half 0..1).
Each core computes, for its batch b and its 4 query heads:
  q = rope(x @ Wq_h'.T)          (gain/(sqrt(hd)*softcap) folded into Wq on host)
  k = rope(x @ Wk.T), v = x @ Wv.T   (single kv head, shared across its 4 q heads)
  pT[k,q] = exp(softcap*tanh(qT.k + causal_bias))   (max-free softmax: softcap
            bounds logits, masked entries get a -8 pre-tanh bias -> e^-30 ~ 0)
  outT_h = (v.T @ pT) / sum_k pT    accumulated in PSUM; denominator via ones-matmul
  partial_out[tok, :] = sum_h outT_h.T @ Wo[:, head cols].T
Host sums the two half-head partials per batch and stacks batches.

v5.3 = v3's ascending-chunk schedule (measured near-zero mid-kernel ACT
idle; v4's reverse-chunk experiments all lost more in the middle than they
saved at the ends) with both ends fixed:
  - head: first task (0,0,0) needs only x(c0)+wk+wq_h0+rope(c0)+mask consts;
    the DMA queues prioritize exactly that prefix, the ACT table set
    preloads via a dummy tanh+exp during the DMA wait, chunk 0-2 K-ropes
    run on DVE (gpsimd is ~4us slower per rope chain), the kp(0)/qp(0,0)
    PSUM casts ride the pre-first-tanh-idle ACT engine, and chunk 0 is
    hand-interleaved (projection h+1 right after scores h) so the four diag
    tasks stream at the DVE rope cadence. First tanh ~18us (v3: ~27us).
  - causal mask folded into the diagonal score accumulation as a -8 bias
    matmul (stationary identity, moving bias pattern): kills 64 DVE muls
    (measured up to ~1.2us each in-place) at +3.4us of N=128 PE matmuls.
  - Wq shipped per-head contiguous ([NHL*128, NKT*HD]) so per-head slices
    DMA as 2KB lines instead of 256B segments.
  - tail: chunk-3 Wo is split by heads: h0+h1 accumulate into fp32 SBUF
    partials as late chunk-3 fillers (their avn are ready mid-chunk); after
    the last exp only the h2+h3 matmuls + a DVE add + split-queue DMA
    remain. Both 512-col output halves of a token block stage into one
    [128,1024] tile and DMA as a single full-row transfer (2KB lines; the
    1KB version left the final writeback packet-rate-bound at ~110GB/s).
    AV lag drops to 1 in chunk 3. ~8.5us of the remaining tail is the
    fixed per-semaphore-zeroing engine postamble (framework-emitted).
All matmuls bf16 (1 cyc/row); scores accumulate fp32 in PSUM; tanh keeps
fp32 until the bf16 exp output. Known-failed: fp8/DoubleRow projections
(even one fp8 tensor exceeds the 2e-2 max-rel-err gate), gpsimd
quad-compress or any gpsimd op feeding a PE matmul (its latency stalls the
in-order PE FIFO), reverse chunk order, fine-grained cost-paced fillers,
PE warmup matmuls, strided wq slices on the scalar DMA queue (256B-segment
transfers clog the queue and delay the rope tables).
"""
import numpy as np
import ml_dtypes

import concourse.bacc as bacc
import concourse.mybir as mybir
import concourse.tile as tile
from concourse.bass_utils import run_bass_kernel_spmd

F32 = mybir.dt.float32
BF16 = mybir.dt.bfloat16
NPBF16 = ml_dtypes.bfloat16

B, T, D = 4, 2048, 1024
NH, NKV, HD = 8, 1, 128
SOFTCAP = 30.0
NHL = 4            # heads per core
CH = 512           # q-chunk size
NCH = T // CH      # 4 chunks
NKT = D // 128     # 8 k-tiles over D
NTT = T // 128     # 16 token tiles

# packed column offsets for diagonal groups: k-block j (visible width
# 512-128j) starts at DOFF[j], arranged so every matmul output stays inside
# one 2KB PSUM bank (512 fp32) with zero padding: bank0 = j0(512),
# bank1 = j1(384) + j3(128), bank2 = j2(256). Total 1280 packed columns.
DOFF = [0, 512, 1024, 896]
NWD = 1280         # diag tanh/exp span
DW = 4 * CH        # score tile width (non-diag)


def _build_nc():
    nc = bacc.Bacc()

    xT = nc.dram_tensor("xT", [D, T], BF16, kind="ExternalInput")
    wqT = nc.dram_tensor("wqT", [NHL * 128, NKT * HD], BF16,
                         kind="ExternalInput")
    wkT = nc.dram_tensor("wkT", [D, HD], BF16, kind="ExternalInput")
    wvT = nc.dram_tensor("wvT", [D, HD], BF16, kind="ExternalInput")
    woT = nc.dram_tensor("woT", [NHL * HD, D], BF16, kind="ExternalInput")
    cc = nc.dram_tensor("cc", [HD, T], BF16, kind="ExternalInput")
    ssw = nc.dram_tensor("ssw", [HD, T], BF16, kind="ExternalInput")
    trib = nc.dram_tensor("trib", [128, 128], BF16, kind="ExternalInput")
    idm = nc.dram_tensor("idm", [128, 128], BF16, kind="ExternalInput")
    onesv = nc.dram_tensor("onesv", [128, 128], BF16, kind="ExternalInput")
    out = nc.dram_tensor("out", [T, D], BF16, kind="ExternalOutput")

    xT_t = xT.rearrange("(kt p) t -> p kt t", p=128)      # [128, 8, 2048]
    wqT_t = wqT.rearrange("(h p) (kt c) -> p h kt c", p=128,
                          kt=NKT)                         # [128, 4, 8, 128]
    wkT_t = wkT.rearrange("(kt p) c -> p kt c", p=128)    # [128, 8, 128]
    wvT_t = wvT.rearrange("(kt p) c -> p kt c", p=128)    # [128, 8, 128]
    woT_t = woT.rearrange("(h p) c -> p h c", p=128)      # [128, 4, 1024]

    with tile.TileContext(nc) as tc:
        with (
            tc.tile_pool(name="persist", bufs=1) as persist,
            tc.tile_pool(name="wpool", bufs=1) as wpool,
            tc.tile_pool(name="qt_pool", bufs=2) as qt_pool,
            tc.tile_pool(name="rope_pool", bufs=3) as rope_pool,
            tc.tile_pool(name="p_pool", bufs=5) as p_pool,
            tc.tile_pool(name="pp_pool", bufs=3) as pp_pool,
            tc.tile_pool(name="t4_pool", bufs=1) as t4_pool,
            tc.tile_pool(name="avn_pool", bufs=12) as avn_pool,
            tc.tile_pool(name="osb_pool", bufs=3) as osb_pool,
            tc.tile_pool(name="o32_pool", bufs=8) as o32_pool,
            tc.tile_pool(name="norm_pool", bufs=2) as norm_pool,
            tc.tile_pool(name="s_pool", bufs=1, space="PSUM") as s_pool,
            tc.tile_pool(name="acc_pool", bufs=1, space="PSUM") as acc_pool,
            tc.tile_pool(name="d_pool", bufs=1, space="PSUM") as d_pool,
            tc.tile_pool(name="pj_pool", bufs=2, space="PSUM") as pj_pool,
        ):
            # --- persistent tiles ---
            wq_sb = wpool.tile([128, NHL, NKT, HD], BF16)
            wk_sb = wpool.tile([128, NKT, HD], BF16)
            wv_sb = wpool.tile([128, NKT, HD], BF16)
            wo_sb = wpool.tile([128, NHL, D], BF16)
            cc_sb = wpool.tile([HD, T], BF16)
            ssw_sb = wpool.tile([HD, T], BF16)
            trib_sb = wpool.tile([128, 128], BF16)
            idm_sb = wpool.tile([128, 128], BF16)
            ones_sb = wpool.tile([128, 128], BF16)
            xT_sb = wpool.tile([128, NKT, T], BF16)
            kT_sb = persist.tile([HD, T], BF16)
            v_sb = persist.tile([128, NTT, HD], BF16)

            # ACT table preload: dummy tanh+exp (same table set) so the
            # ~2.7us ACT_TABLE_LOAD runs during the DMA wait, not inside
            # the first real tanh.
            warm = wpool.tile([1, 16], F32)
            nc.vector.memset(warm[:], 0.0)
            warm2 = wpool.tile([1, 16], F32)
            nc.scalar.activation(warm2[:], warm[:],
                                 mybir.ActivationFunctionType.Tanh)
            nc.scalar.activation(warm2[:], warm[:],
                                 mybir.ActivationFunctionType.Exp)

            # --- DMA priorities ---
            # First task (0,0,0) needs x(c0), wk, wq-h0, cc/ssw(c0) and the
            # mask consts; everything else follows in chunk order. The two
            # HWDGE queues stripe over the same 16 engines, so the critical
            # prefix is split across both; the scalar queue finishes its
            # issues by ~12us so the Scalar engine is pure ACT afterwards.
            c0 = slice(0, CH)
            c1 = slice(CH, 2 * CH)
            c2 = slice(2 * CH, 3 * CH)
            c3 = slice(3 * CH, T)
            nc.sync.dma_start(xT_sb[:, 0:1, c0], xT_t[:, 0:1, c0])
            nc.scalar.dma_start(wk_sb[:], wkT_t)
            nc.sync.dma_start(xT_sb[:, 1:4, c0], xT_t[:, 1:4, c0])
            nc.scalar.dma_start(cc_sb[:, c0], cc[:, c0])
            nc.scalar.dma_start(ssw_sb[:, c0], ssw[:, c0])
            nc.sync.dma_start(xT_sb[:, 4:8, c0], xT_t[:, 4:8, c0])
            nc.scalar.dma_start(trib_sb[:], trib[:])
            nc.scalar.dma_start(idm_sb[:], idm[:])
            nc.scalar.dma_start(ones_sb[:], onesv[:])
            nc.sync.dma_start(wq_sb[:, 0], wqT_t[:, 0])
            nc.scalar.dma_start(wv_sb[:], wvT_t)
            nc.scalar.dma_start(cc_sb[:, c1], cc[:, c1])
            nc.scalar.dma_start(ssw_sb[:, c1], ssw[:, c1])
            nc.sync.dma_start(xT_sb[:, :, c1], xT_t[:, :, c1])
            nc.sync.dma_start(wq_sb[:, 1], wqT_t[:, 1])
            nc.sync.dma_start(xT_sb[:, :, c2], xT_t[:, :, c2])
            nc.sync.dma_start(wq_sb[:, 2], wqT_t[:, 2])
            nc.sync.dma_start(wq_sb[:, 3], wqT_t[:, 3])
            nc.sync.dma_start(cc_sb[:, c2], cc[:, c2])
            nc.sync.dma_start(ssw_sb[:, c2], ssw[:, c2])
            nc.sync.dma_start(xT_sb[:, :, c3], xT_t[:, :, c3])
            nc.sync.dma_start(cc_sb[:, c3], cc[:, c3])
            nc.sync.dma_start(ssw_sb[:, c3], ssw[:, c3])
            nc.sync.dma_start(wo_sb[:], woT_t)

            def rope_to(dst_ap, src_ps, c, eng=None, cast_eng=None):
                """dst = rope(src) for a [128, CH] chunk at token offset c*CH.

                Partition half-swap must go through tensor_copy (TT ops need
                aligned partitions). cast_eng handles the PSUM fp32 -> bf16
                read (DVE normally; ACT for the pre-first-tanh units), eng
                (DVE for latency-critical Q/chunk-0-K ropes, gpsimd for the
                pre-emitted later K ropes) runs the mul/add."""
                if eng is None:
                    eng = nc.vector
                csl = slice(c * CH, (c + 1) * CH)
                qb = rope_pool.tile([128, CH], BF16, tag="qb", name="qb")
                if cast_eng is nc.scalar:
                    nc.scalar.copy(qb[:], src_ps[:])
                else:
                    nc.vector.tensor_copy(qb[:], src_ps[:])
                swp = rope_pool.tile([128, CH], BF16, tag="swp", name="swp")
                nc.vector.tensor_copy(swp[0:64, :], qb[64:128, :])
                nc.vector.tensor_copy(swp[64:128, :], qb[0:64, :])
                m1 = rope_pool.tile([128, CH], BF16, tag="m1", name="m1")
                eng.tensor_mul(m1[:], qb[:], cc_sb[:, csl])
                m2 = rope_pool.tile([128, CH], BF16, tag="m2", name="m2")
                eng.tensor_mul(m2[:], swp[:], ssw_sb[:, csl])
                eng.tensor_add(dst_ap, m1[:], m2[:])

            # ---- filler units (pure-PE work scheduled into ACT-bound gaps) ----
            qt_tiles = {}     # c -> qt tile [HD, NHL, CH]

            def qp_unit(c, h):
                csl = slice(c * CH, (c + 1) * CH)
                if h == 0:
                    qt_tiles[c] = qt_pool.tile([HD, NHL, CH], BF16, tag="qt",
                                               name="qt")
                q_ps = pj_pool.tile([128, CH], F32, tag="pj", name="q_ps")
                for kt in range(NKT):
                    nc.tensor.matmul(
                        q_ps[0:HD, :], wq_sb[:, h, kt, :],
                        xT_sb[:, kt, csl], start=(kt == 0), stop=(kt == NKT - 1))
                cast_eng = nc.scalar if (c == 0 or (c == 1 and h == 0)) \
                    else None
                rope_to(qt_tiles[c][:, h, :], q_ps[0:HD, :], c,
                        cast_eng=cast_eng)

            def kp_unit(c):
                csl = slice(c * CH, (c + 1) * CH)
                k_ps = pj_pool.tile([128, CH], F32, tag="pj", name="k_ps")
                for kt in range(NKT):
                    nc.tensor.matmul(k_ps[0:HD, :], wk_sb[:, kt, :],
                                     xT_sb[:, kt, csl],
                                     start=(kt == 0), stop=(kt == NKT - 1))
                # K ropes gate the next chunk's first tasks at each chunk
                # boundary; they stay on DVE (chunk-0's with an ACT cast,
                # pre-first-tanh) except kp(3), emitted ~a chunk ahead,
                # where gpsimd's slowness is hidden.
                if c <= 1:
                    rope_to(kT_sb[:, csl], k_ps[0:HD, :], c,
                            cast_eng=nc.scalar)
                elif c == 3:
                    rope_to(kT_sb[:, csl], k_ps[0:HD, :], c, eng=nc.gpsimd)
                else:
                    rope_to(kT_sb[:, csl], k_ps[0:HD, :], c)

            def vp_unit(c, tt):
                # V directly as [tok, hd]: x-tile stationary, wv moving.
                tsl = slice((c * 4 + tt) * 128, (c * 4 + tt + 1) * 128)
                v_ps = pj_pool.tile([128, CH], F32, tag="pj", name="v_ps")
                for kt in range(NKT):
                    nc.tensor.matmul(v_ps[:, 0:HD], xT_sb[:, kt, tsl],
                                     wv_sb[:, kt, :],
                                     start=(kt == 0), stop=(kt == NKT - 1))
                nc.vector.tensor_copy(v_sb[:, c * 4 + tt, :], v_ps[:, 0:HD])

            avn_tiles = {}    # (c, h) -> avn tile
            o32_tiles = {}    # u -> chunk-3 h0+h1 fp32 partial (SBUF)
            osb_pairs = {}    # (c, tt) -> [128, 2*CH] staging tile

            def _osb(c, tt, dc):
                # the two 512-col halves of a token block stage into ONE
                # [128, 1024] tile; the odd half issues a single full-row
                # DMA (2KB lines: the 1KB-line version left the final
                # writeback packet-rate-bound at ~110GB/s)
                if dc == 0:
                    osb_pairs[(c, tt)] = osb_pool.tile(
                        [128, 2 * CH], BF16, tag="osb", name="o_sb")
                return osb_pairs[(c, tt)]

            def wo_unit(c, u):
                tt, dc = u // 2, u % 2
                o_ps = pj_pool.tile([128, CH], F32, tag="pj", name="o_ps")
                for h in range(NHL):
                    nc.tensor.matmul(
                        o_ps[:], avn_tiles[(c, h)][:, tt * 128:(tt + 1) * 128],
                        wo_sb[:, h, dc * CH:(dc + 1) * CH],
                        start=(h == 0), stop=(h == NHL - 1))
                # DMA can't source PSUM (nor can gpsimd): stage on DVE.
                # bf16 halves the out DMA; host sums partials in fp32.
                o_sb = _osb(c, tt, dc)
                nc.vector.tensor_copy(o_sb[:, dc * CH:(dc + 1) * CH], o_ps[:])
                if dc == 1:
                    nc.sync.dma_start(
                        out[c * CH + tt * 128: c * CH + (tt + 1) * 128, :],
                        o_sb[:])

            def wo3a_unit(u):
                # chunk-3 Wo, heads 0+1 only: accumulate into an fp32 SBUF
                # partial during chunk 3 (avn(3,0/1) are ready mid-chunk),
                # so the post-last-exp tail holds only the h2+h3 half.
                tt, dc = u // 2, u % 2
                o_ps = pj_pool.tile([128, CH], F32, tag="pj", name="o_ps")
                for h in (0, 1):
                    nc.tensor.matmul(
                        o_ps[:], avn_tiles[(3, h)][:, tt * 128:(tt + 1) * 128],
                        wo_sb[:, h, dc * CH:(dc + 1) * CH],
                        start=(h == 0), stop=(h == 1))
                o32 = o32_pool.tile([128, CH], F32, tag="o32", name="o32")
                nc.vector.tensor_copy(o32[:], o_ps[:])
                o32_tiles[u] = o32

            def wo3b_unit(u):
                tt, dc = u // 2, u % 2
                o_ps = pj_pool.tile([128, CH], F32, tag="pj", name="o_ps")
                for h in (2, 3):
                    nc.tensor.matmul(
                        o_ps[:], avn_tiles[(3, h)][:, tt * 128:(tt + 1) * 128],
                        wo_sb[:, h, dc * CH:(dc + 1) * CH],
                        start=(h == 2), stop=(h == 3))
                o_sb = _osb(3, tt, dc)
                # fp32 partial + fp32 PSUM -> bf16 out, one DVE add; the
                # tail splits the full-row DMAs across both queues.
                nc.vector.tensor_add(o_sb[:, dc * CH:(dc + 1) * CH], o_ps[:],
                                     o32_tiles[u][:])
                if dc == 1:
                    eng = nc.scalar if tt % 2 == 1 else nc.sync
                    eng.dma_start(
                        out[3 * CH + tt * 128: 3 * CH + (tt + 1) * 128, :],
                        o_sb[:])

            # ---- filler scheduling ----
            emitted = set()

            def emit_unit(u):
                if u in emitted:
                    return
                emitted.add(u)
                kind = u[0]
                if kind == "qp":
                    qp_unit(u[1], u[2])
                elif kind == "kp":
                    kp_unit(u[1])
                elif kind == "vp":
                    vp_unit(u[1], u[2])
                elif kind == "wo":
                    wo_unit(u[1], u[2])
                elif kind == "wo3a":
                    wo3a_unit(u[1])
                elif kind == "wo3b":
                    wo3b_unit(u[1])

            # per-chunk filler lists. Only qp(c+1,0)/kp(c+1) cross chunk
            # boundaries; vp(c) and qp(c,h>=1) stay inside chunk c (forced
            # just-in-time), and Wo shifts late into the ACT-heavy chunks
            # 2 and 3 to match the causal skew of attention work. wo3a units
            # sit at the end of chunk 3's list, after avn(3,0/1) exist.
            fillers = {}
            fillers[0] = ([("qp", 0, 2), ("qp", 0, 3), ("kp", 1)]
                          + [("vp", 0, tt) for tt in range(4)]
                          + [("qp", 1, 0)])
            fillers[1] = ([("vp", 1, tt) for tt in range(4)]
                          + [("kp", 2), ("qp", 2, 0)])
            fillers[2] = ([("vp", 2, tt) for tt in range(4)]
                          + [("wo", 0, u) for u in range(8)]
                          + [("kp", 3), ("qp", 3, 0)])
            fillers[3] = ([("vp", 3, tt) for tt in range(4)]
                          + [("wo", 1, u) for u in range(8)]
                          + [("wo", 2, u) for u in range(8)]
                          + [("wo3a", u) for u in range(8)])

            # ---- attention task machinery ----
            pend = []       # lagged AV work queue: (c, h, g, p4_tile)
            AV_LAG = 2      # tasks between exp(i) and its AV consumption
            prefill = {}    # (c, h, g) -> p4 computed early (ACT-idle fill)
            head_acc = {}   # (c, h) -> (av_ps, d_ps), allocated at g == 0

            def emit_av(c, h, g, p4):
                """AV + quad-compress + ones-matmul for task (c,h,g); the
                consuming accumulators live across the head's groups."""
                diag = g == c
                for tt in range(4):
                    emit_unit(("vp", g, tt))
                if g == 0:
                    av_ps = acc_pool.tile([HD, CH], F32, tag="av", name="av_ps")
                    d_ps = d_pool.tile([128, CH], F32, tag="d", name="d_ps")
                    head_acc[(c, h)] = (av_ps, d_ps)
                av_ps, d_ps = head_acc[(c, h)]
                for j in range(4):
                    kb = 4 * g + j
                    if diag:
                        lo, po = 128 * j, DOFF[j]
                        w = CH - lo
                        nc.tensor.matmul(av_ps[:, lo:CH], v_sb[:, kb, :],
                                         p4[:, po:po + w],
                                         start=(kb == 0),
                                         stop=(g == c and j == 3))
                    else:
                        nc.tensor.matmul(av_ps[:], v_sb[:, kb, :],
                                         p4[:, j * CH:(j + 1) * CH],
                                         start=(kb == 0), stop=False)
                # quad-compress for the denominator: 3 adds -> 1 ones-MM
                ppq = pp_pool.tile([128, CH], BF16, tag="ppq", name="ppq")
                if diag:
                    nc.vector.tensor_copy(ppq[:], p4[:, 0:CH])
                    for j in range(1, 4):
                        lo = 128 * j
                        nc.vector.tensor_add(
                            ppq[:, lo:CH], ppq[:, lo:CH],
                            p4[:, DOFF[j]:DOFF[j] + (CH - lo)])
                else:
                    ppa = pp_pool.tile([128, CH], BF16, tag="ppa", name="ppa")
                    nc.vector.tensor_add(ppa[:], p4[:, 0:CH], p4[:, CH:2 * CH])
                    ppb = pp_pool.tile([128, CH], BF16, tag="ppb", name="ppb")
                    nc.vector.tensor_add(ppb[:], p4[:, 2 * CH:3 * CH],
                                         p4[:, 3 * CH:4 * CH])
                    nc.vector.tensor_add(ppq[:], ppa[:], ppb[:])
                nc.tensor.matmul(d_ps[:], ones_sb[:], ppq[:],
                                 start=(g == 0), stop=(g == c))
                if g == c:
                    # head (c,h) complete: normalize
                    dinv = norm_pool.tile([128, CH], F32, tag="dinv",
                                          name="dinv")
                    nc.vector.reciprocal_approx_fast(dinv[:], d_ps[:])
                    avn = avn_pool.tile([HD, CH], BF16, tag="avn", name="avn")
                    nc.vector.tensor_mul(avn[:], av_ps[:], dinv[:])
                    avn_tiles[(c, h)] = avn

            def emit_scores(c, h, g):
                """scores (+ causal bias on diag) -> tanh -> exp."""
                diag = g == c
                emit_unit(("kp", g))
                qt = qt_tiles[c]
                s_t = s_pool.tile([128, DW], F32, tag="s", name="s_t")
                t4 = t4_pool.tile([128, DW], F32, tag="t4", name="t4")
                p4 = p_pool.tile([128, DW], BF16, tag="p4", name="p4")
                if diag:
                    # causal mask folded into the score accumulation: a -8
                    # bias matmul (stationary identity, moving trib) onto
                    # the first 128 q-cols of each block saturates tanh to
                    # -1, so masked weights exp to e^-30 ~ 0: no DVE muls.
                    for j in range(4):
                        kb = 4 * g + j
                        lo, po = 128 * j, DOFF[j]
                        w = CH - lo
                        nc.tensor.matmul(
                            s_t[:, po:po + w],
                            kT_sb[:, kb * 128:(kb + 1) * 128],
                            qt[:, h, lo:CH], start=True, stop=False)
                        nc.tensor.matmul(
                            s_t[:, po:po + 128], idm_sb[:], trib_sb[:],
                            start=False, stop=True, skip_group_check=True)
                    nw = NWD
                else:
                    for j in range(4):
                        kb = 4 * g + j
                        nc.tensor.matmul(
                            s_t[:, j * CH:(j + 1) * CH],
                            kT_sb[:, kb * 128:(kb + 1) * 128],
                            qt[:, h, :], start=True, stop=True)
                    nw = DW
                nc.scalar.activation(t4[:, 0:nw], s_t[:, 0:nw],
                                     mybir.ActivationFunctionType.Tanh)
                nc.scalar.activation(p4[:, 0:nw], t4[:, 0:nw],
                                     mybir.ActivationFunctionType.Exp,
                                     scale=SOFTCAP)
                return p4

            # ---- main schedule ----
            # prologue: just enough for the first task. K first: wk lands
            # early on the scalar queue, so the PE clock ramp starts earlier.
            emit_unit(("kp", 0))
            emit_unit(("qp", 0, 0))

            # chunk 0 hand-ordered: each next head's projection enters the
            # PE FIFO right after the previous head's scores, so the four
            # diag tasks stream at the DVE rope cadence with no filler
            # bundle in front of the next scores.
            c0units = [[("qp", 0, 1)], [("qp", 0, 2)], [("qp", 0, 3)],
                       [("kp", 1), ("vp", 0, 0), ("vp", 0, 1)]]
            for h in range(NHL):
                p4 = emit_scores(0, h, 0)
                pend.append((0, h, 0, p4))
                while len(pend) > AV_LAG:
                    emit_av(*pend.pop(0))
                for u in c0units[h]:
                    emit_unit(u)
            emit_unit(("qp", 1, 0))
            prefill[(1, 0, 0)] = emit_scores(1, 0, 0)

            for c in range(1, NCH):
                if c >= 1:
                    # cross-boundary fillers must have landed (kp(c)/qp(c,0))
                    for u in fillers[c - 1]:
                        emit_unit(u)
                flist = fillers[c]
                # drain fillers one task early so chunk boundaries are clean
                ntasks = max(1, NHL * (c + 1) - 1)
                nf = len(flist)
                ti = 0
                lag = AV_LAG if c < NCH - 1 else 1
                for h in range(NHL):
                    emit_unit(("qp", c, h))
                    for g in range(c + 1):
                        if (c, h, g) in prefill:
                            p4 = prefill.pop((c, h, g))
                        else:
                            p4 = emit_scores(c, h, g)
                        pend.append((c, h, g, p4))
                        while len(pend) > lag:
                            emit_av(*pend.pop(0))
                        if g == 0 and h + 1 < NHL:
                            # project the next head now: its rope latency
                            # hides under this head's ACT work
                            emit_unit(("qp", c, h + 1))
                        # spread this chunk's fillers evenly across tasks
                        lo = min(nf, (ti * nf) // ntasks)
                        hi = min(nf, ((ti + 1) * nf) // ntasks)
                        for u in flist[lo:hi]:
                            emit_unit(u)
                        ti += 1
                # chunks 0/1 end ACT-idle (proj-heavy, little attention):
                # prefill the next chunk's first non-diag scores/tanh/exp
                # there; their AV stays in place (p4 carries across)
                if c + 1 < NCH and c <= 1:
                    emit_unit(("qp", c + 1, 0))
                    for g in range(min(2, c + 1)):
                        prefill[(c + 1, 0, g)] = emit_scores(c + 1, 0, g)
            while pend:
                emit_av(*pend.pop(0))
            for u in fillers[NCH - 1]:
                emit_unit(u)
            for u in range(8):
                emit_unit(("wo3b", u))

    nc.compile()
    return nc


_CACHED_NC = None


def _get_nc():
    global _CACHED_NC
    if _CACHED_NC is None:
        _CACHED_NC = _build_nc()
    return _CACHED_NC


def _host_inputs(x, Wq, Wk, Wv, Wo, qk_gain, cos, sin):
    """Build the 8 per-core input maps (bf16 matmul operands)."""
    x = np.asarray(x, np.float32)
    Wq = np.asarray(Wq, np.float32)
    Wk = np.asarray(Wk, np.float32)
    Wv = np.asarray(Wv, np.float32)
    Wo = np.asarray(Wo, np.float32)
    qk_gain = np.asarray(qk_gain, np.float32)
    cos = np.asarray(cos, np.float32)
    sin = np.asarray(sin, np.float32)

    scale = 1.0 / (np.sqrt(HD) * SOFTCAP)
    # Fold per-head gain and softcap scale into Wq rows.
    Wq_s = Wq * (qk_gain[:, None].repeat(HD, 1).reshape(NH * HD, 1) * scale)

    wkT = np.ascontiguousarray(Wk.T.astype(NPBF16))
    wvT = np.ascontiguousarray(Wv.T.astype(NPBF16))
    cosT = cos.T  # [64, T]
    sinT = sin.T
    cc = np.ascontiguousarray(np.concatenate([cosT, cosT], 0).astype(NPBF16))
    # m2 = swap(q) * ssw with swap done via copies: ssw = [-sin; sin]
    ssw = np.ascontiguousarray(np.concatenate([-sinT, sinT], 0).astype(NPBF16))

    # causal bias for the diagonal 128-blocks: 0 where visible (qq >= kk),
    # -8 where masked -- saturates tanh pre-exp so masked weights ~ e^-30.
    kk = np.arange(128)
    trib = np.where(kk[None, :] >= kk[:, None], 0.0, -8.0).astype(NPBF16)
    idm = np.eye(128, dtype=NPBF16)
    onesv = np.ones((128, 128), NPBF16)

    xTs = [np.ascontiguousarray(x[b].T.astype(NPBF16)) for b in range(B)]
    in_maps = []
    for core in range(8):
        b, hh = divmod(core, 2)
        h0 = hh * NHL
        # per head h: [p, kt, c] with (kt,c) contiguous per partition row
        wqT = np.ascontiguousarray(np.stack(
            [Wq_s[(h0 + h) * HD:(h0 + h + 1) * HD, :]
             .reshape(HD, NKT, 128).transpose(2, 1, 0).reshape(128, NKT * HD)
             for h in range(NHL)], axis=0)
            .reshape(NHL * 128, NKT * HD).astype(NPBF16))
        woT = np.ascontiguousarray(
            Wo[:, h0 * HD:(h0 + NHL) * HD].T.astype(NPBF16))
        in_maps.append({
            "xT": xTs[b], "wqT": wqT, "wkT": wkT, "wvT": wvT, "woT": woT,
            "cc": cc, "ssw": ssw, "trib": trib, "idm": idm,
            "onesv": onesv,
        })
    return in_maps


def kernel(x, Wq, Wk, Wv, Wo, qk_gain, cos, sin, _trace=False):
    in_maps = _host_inputs(x, Wq, Wk, Wv, Wo, qk_gain, cos, sin)
    nc = _get_nc()
    res = run_bass_kernel_spmd(nc, in_maps, core_ids=list(range(8)),
                               trace=_trace)
    out = np.empty((B, T, D), np.float32)
    for b in range(B):
        out[b] = (res.results[2 * b]["out"].astype(np.float32)
                  + res.results[2 * b + 1]["out"].astype(np.float32))
    if _trace:
        kernel.last_exec_time_ns = res.exec_time_ns
        kernel.last_results = res
    return out
